# revision 7
# baseline (speedup 1.0000x reference)
"""Bass/Trainium2 kernel for nn_BasicLSTM: LayerNorm-LSTM cell, B=16384, dims 1024.

Strategy (data-parallel over 8 cores, batch-sharded, weights replicated):
  Per core: batch 2048. Feature-major gate computation:
    - LN(x), LN(h_prev) computed batch-major on device (DVE free-dim reduce),
      normalized output cast to fp16 and DMA-transposed into actT [feat, batch].
      LN scale/bias are folded into the gate weights/biases on the host.
    - Gate matmuls: out[gate_feat 128, batch 512] = WgT_tile.T @ actT_tile,
      fp16 x fp16, K=2048 (16 k-tiles), PSUM fp32 accumulation.
      Per-gate-feature biases fused into the ACT nonlinearity (per-partition bias).
    - Elementwise c_t/h_t in feature-major layout (c_prev host-transposed).
    - y matmul from fp16 h_t (feature-major already), erf-Gelu via ACT.
    - Outputs written feature-major [1024, 2048]; host transposes back.
"""
import os
import numpy as np

B, IN, H, OUT = 16384, 1024, 1024, 1024
NCORES = 8
BC = B // NCORES          # 2048 rows per core
EPS = 1e-5
P = 128
KT_G = 16                 # gate contraction k-tiles (2048 / 128)
KT_Y = 8                  # y contraction k-tiles (1024 / 128)
NB4 = BC // 512           # batch 512-blocks per core = 4
NOC = 8                   # gate-feature 128-chunks = 1024/128

# Use erf-composed gelu unless env overrides (ACT Gelu table may be approximate).
GELU_ERF = os.environ.get("LSTM_GELU_ERF", "1") == "1"

_CACHE = {}

_last_results = None  # BassKernelResults of the most recent run (for test harness)


def _build_nc():
    import concourse.tile as tile
    from concourse import bacc, mybir
    from concourse._compat import get_trn_type

    f32 = mybir.dt.float32
    f16 = mybir.dt.float16
    AF = mybir.ActivationFunctionType
    ALU = mybir.AluOpType

    nc = bacc.Bacc(get_trn_type() or "TRN2", target_bir_lowering=False)

    x_d = nc.dram_tensor("x", (BC, IN), f32, kind="ExternalInput")
    h_d = nc.dram_tensor("h_prev", (BC, H), f32, kind="ExternalInput")
    cT_d = nc.dram_tensor("c_prevT", (H, BC), f32, kind="ExternalInput")
    wg_d = nc.dram_tensor("Wg", (NOC, KT_G, P, 512), f16, kind="ExternalInput")
    bg_d = nc.dram_tensor("bg", (P, 32), f32, kind="ExternalInput")
    why_d = nc.dram_tensor("WhyT", (KT_Y, P, OUT), f16, kind="ExternalInput")
    by_d = nc.dram_tensor("by_col", (P, 8), f32, kind="ExternalInput")

    ct_o = nc.dram_tensor("c_tT", (H, BC), f32, kind="ExternalOutput")
    ht_o = nc.dram_tensor("h_tT", (H, BC), f32, kind="ExternalOutput")
    yt_o = nc.dram_tensor("y_tT", (OUT, BC), f32, kind="ExternalOutput")

    with tile.TileContext(nc) as tc:
        with (
            tc.tile_pool(name="const", bufs=1) as const,
            tc.tile_pool(name="actT", bufs=NB4) as actT_pool,
            tc.tile_pool(name="h16", bufs=NB4) as h16_pool,
            tc.tile_pool(name="wg", bufs=2) as wg_pool,
            tc.tile_pool(name="why", bufs=1) as why_pool,
            tc.tile_pool(name="xh", bufs=2) as xh_pool,
            tc.tile_pool(name="sqp", bufs=2) as sq_pool,
            tc.tile_pool(name="lnst", bufs=6) as lnst_pool,
            tc.tile_pool(name="xn16", bufs=2) as xn16_pool,
            tc.tile_pool(name="stg", bufs=2) as stg_pool,
            tc.tile_pool(name="gate", bufs=2) as gate_pool,
            tc.tile_pool(name="cpv", bufs=2) as cpv_pool,
            tc.tile_pool(name="outs", bufs=2) as outs_pool,
            tc.tile_pool(name="ps", bufs=8, space="PSUM") as ps_pool,
        ):
            bg_sb = const.tile([P, 32], f32)
            nc.sync.dma_start(bg_sb[:], bg_d[:])
            by_sb = const.tile([P, 8], f32)
            nc.sync.dma_start(by_sb[:], by_d[:])

            # Resident fp16 transposed activations: 4 b4-blocks of [feat_p, k, batch512]
            actT = [actT_pool.tile([P, KT_G, 512], f16, tag="actT", name=f"actT{i}") for i in range(NB4)]
            # Resident fp16 feature-major h_t for the y matmul
            h16 = [h16_pool.tile([P, KT_Y, 512], f16, tag="h16", name=f"h16_{i}") for i in range(NB4)]

            # ---- Phase 1: LN(x), LN(h_prev) batch-major; cast fp16; transpose ----
            def ln_panel(src_d, m, ko_off):
                xp = xh_pool.tile([P, 1024], f32, tag="xh")
                nc.gpsimd.dma_start(xp[:], src_d[m * P:(m + 1) * P, :])
                s1 = lnst_pool.tile([P, 1], f32, tag="s1")
                s2 = lnst_pool.tile([P, 1], f32, tag="s2")
                sq = sq_pool.tile([P, 1024], f32, tag="sq")
                nc.scalar.activation(sq[:], xp[:], AF.Square, accum_out=s2[:])
                nc.scalar.activation(sq[:], xp[:], AF.Identity, accum_out=s1[:])
                mu = lnst_pool.tile([P, 1], f32, tag="mu")
                nc.vector.tensor_scalar_mul(mu[:], s1[:], 1.0 / 1024.0)
                ve = lnst_pool.tile([P, 1], f32, tag="ve")
                nc.vector.tensor_scalar(ve[:], s2[:], 1.0 / 1024.0, EPS, ALU.mult, ALU.add)
                m2 = lnst_pool.tile([P, 1], f32, tag="m2")
                nc.vector.tensor_mul(out=m2[:], in0=mu[:], in1=mu[:])
                nc.vector.tensor_tensor(ve[:], ve[:], m2[:], ALU.subtract)
                sd = lnst_pool.tile([P, 1], f32, tag="sd")
                nc.scalar.sqrt(sd[:], ve[:])
                rstd = lnst_pool.tile([P, 1], f32, tag="rstd")
                nc.vector.reciprocal(rstd[:], sd[:])
                mur = lnst_pool.tile([P, 1], f32, tag="mur")
                nc.vector.tensor_mul(out=mur[:], in0=mu[:], in1=rstd[:])
                z16 = xn16_pool.tile([P, 1024], f16, tag="z16")
                nc.vector.tensor_scalar(z16[:], xp[:], rstd[:, 0:1], mur[:, 0:1],
                                        ALU.mult, ALU.subtract)
                # XBAR transpose needs a contiguous destination; bounce through a
                # contiguous staging tile, then strided-copy into actT.
                st = stg_pool.tile([P, 8, P], f16, tag="st")
                nc.sync.dma_start_transpose(st[:], z16[:])
                b4, mloc = divmod(m, 4)
                nc.gpsimd.dma_start(
                    actT[b4][:, ko_off:ko_off + 8, mloc * P:(mloc + 1) * P], st[:])

            for b4 in range(NB4):
                for mloc in range(4):
                    ln_panel(x_d, b4 * 4 + mloc, 0)
                for mloc in range(4):
                    ln_panel(h_d, b4 * 4 + mloc, 8)

            # ---- Phase 2: gates (oc-major, weights streamed once) ----
            why = why_pool.tile([P, KT_Y, 1024], f16, tag="why")
            nc.gpsimd.dma_start(why[:], why_d.rearrange("ko p c -> p ko c"))
            for oc in range(NOC):
                wg = wg_pool.tile([P, KT_G, 512], f16, tag="wg")
                nc.gpsimd.dma_start(wg[:], wg_d[oc].rearrange("ko p c -> p ko c"))
                for b4 in range(NB4):
                    pss = []
                    for g in range(4):
                        ps = ps_pool.tile([P, 512], f32, tag="ps")
                        for k in range(KT_G):
                            nc.tensor.matmul(ps[:], wg[:, k, g * P:(g + 1) * P],
                                             actT[b4][:, k, :],
                                             start=(k == 0), stop=(k == KT_G - 1))
                        pss.append(ps)
                    f_sb = gate_pool.tile([P, 512], f32, tag="f")
                    nc.scalar.activation(f_sb[:], pss[0][:], AF.Sigmoid,
                                         bias=bg_sb[:, oc * 4 + 0:oc * 4 + 1])
                    i_sb = gate_pool.tile([P, 512], f32, tag="i")
                    nc.scalar.activation(i_sb[:], pss[1][:], AF.Sigmoid,
                                         bias=bg_sb[:, oc * 4 + 1:oc * 4 + 2])
                    o_sb = gate_pool.tile([P, 512], f32, tag="o")
                    nc.scalar.activation(o_sb[:], pss[2][:], AF.Sigmoid,
                                         bias=bg_sb[:, oc * 4 + 2:oc * 4 + 3])
                    cp_sb = gate_pool.tile([P, 512], f32, tag="cp")
                    nc.scalar.activation(cp_sb[:], pss[3][:], AF.Tanh,
                                         bias=bg_sb[:, oc * 4 + 3:oc * 4 + 4])

                    cpv = cpv_pool.tile([P, 512], f32, tag="cpv")
                    nc.gpsimd.dma_start(cpv[:], cT_d[oc * P:(oc + 1) * P,
                                                     b4 * 512:(b4 + 1) * 512])
                    nc.vector.tensor_mul(out=f_sb[:], in0=f_sb[:], in1=cpv[:])
                    nc.vector.tensor_mul(out=i_sb[:], in0=i_sb[:], in1=cp_sb[:])
                    ct = outs_pool.tile([P, 512], f32, tag="ct")
                    nc.vector.tensor_add(out=ct[:], in0=f_sb[:], in1=i_sb[:])
                    nc.sync.dma_start(ct_o[oc * P:(oc + 1) * P,
                                           b4 * 512:(b4 + 1) * 512], ct[:])
                    nc.scalar.activation(cp_sb[:], ct[:], AF.Tanh)
                    nc.vector.tensor_mul(out=o_sb[:], in0=o_sb[:], in1=cp_sb[:])
                    nc.sync.dma_start(ht_o[oc * P:(oc + 1) * P,
                                           b4 * 512:(b4 + 1) * 512], o_sb[:])
                    nc.vector.tensor_copy(h16[b4][:, oc, :], o_sb[:])

            # ---- Phase 3: y ----
            for b4 in range(NB4):
                for n in range(OUT // P):
                    ps = ps_pool.tile([P, 512], f32, tag="ps")
                    for k in range(KT_Y):
                        nc.tensor.matmul(ps[:], why[:, k, n * P:(n + 1) * P],
                                         h16[b4][:, k, :],
                                         start=(k == 0), stop=(k == KT_Y - 1))
                    yt = outs_pool.tile([P, 512], f32, tag="yt")
                    u = gate_pool.tile([P, 512], f32, tag="u")
                    nc.scalar.activation(u[:], ps[:], AF.Identity,
                                         bias=by_sb[:, n:n + 1])
                    e = gate_pool.tile([P, 512], f32, tag="e")
                    nc.scalar.activation(e[:], u[:], AF.Erf,
                                         scale=float(1.0 / np.sqrt(2.0)))
                    nc.vector.tensor_scalar(e[:], e[:], 0.5, 0.5, ALU.mult, ALU.add)
                    nc.vector.tensor_mul(out=yt[:], in0=u[:], in1=e[:])
                    nc.sync.dma_start(yt_o[n * P:(n + 1) * P,
                                           b4 * 512:(b4 + 1) * 512], yt[:])

    nc.compile()
    return nc


def _host_prep(inputs):
    """Build replicated device-side weight/bias arrays from the full inputs."""
    f32 = np.float32
    ln_w = inputs["ln_w"].astype(f32)
    ln_b = inputs["ln_b"].astype(f32)
    lnh_w = inputs["lnh_w"].astype(f32)
    lnh_b = inputs["lnh_b"].astype(f32)

    Wx = np.concatenate([inputs["Wxf"], inputs["Wxi"], inputs["Wxo"], inputs["Wxc"]],
                        axis=0).astype(f32)          # [4096, IN] (out, in)
    Wh = np.concatenate([inputs["Whf"], inputs["Whi"], inputs["Who"], inputs["Whc"]],
                        axis=0).astype(f32)          # [4096, H]
    bg = np.concatenate([inputs["bf"], inputs["bi"], inputs["bo"], inputs["bc"]]).astype(f32)

    # Fold LN affine into weights / bias:
    #   xn @ Wx.T = z_x @ (ln_w * Wx).T + ln_b @ Wx.T
    Wx_eff = Wx * ln_w[None, :]
    Wh_eff = Wh * lnh_w[None, :]
    bias_eff = bg + Wx @ ln_b + Wh @ lnh_b           # [4096]

    Wcat = np.concatenate([Wx_eff.T, Wh_eff.T], axis=0)  # [2048 in', 4096 out]
    # Device layout [oc, ko, p, (g,j)] with 512-col groups [f_oc|i_oc|o_oc|c_oc]
    Wg = (Wcat.reshape(KT_G, P, 4, NOC, P)
              .transpose(3, 0, 1, 2, 4)
              .reshape(NOC, KT_G, P, 512)
              .astype(np.float16))
    bg_dev = np.ascontiguousarray(
        bias_eff.reshape(4, NOC, P).transpose(2, 1, 0).reshape(P, 32)).astype(f32)

    WhyT = np.ascontiguousarray(inputs["Why"].astype(f32).T)   # [H, OUT]
    Why_dev = WhyT.reshape(KT_Y, P, OUT).astype(np.float16)
    by_dev = np.ascontiguousarray(inputs["by"].astype(f32).reshape(8, P).T)  # [P, 8]

    return Wg, bg_dev, Why_dev, by_dev


def kernel(**inputs):
    global _last_results
    from concourse.bass_utils import run_bass_kernel_spmd

    if "nc" not in _CACHE:
        _CACHE["nc"] = _build_nc()
    nc = _CACHE["nc"]

    Wg, bg_dev, Why_dev, by_dev = _host_prep(inputs)

    x = np.ascontiguousarray(inputs["x"], dtype=np.float32).reshape(NCORES, BC, IN)
    hp = np.ascontiguousarray(inputs["h_prev"], dtype=np.float32).reshape(NCORES, BC, H)
    cpT = np.ascontiguousarray(
        np.asarray(inputs["c_prev"], dtype=np.float32).reshape(NCORES, BC, H)
        .transpose(0, 2, 1))                          # [NCORES, H, BC]

    in_maps = []
    for c in range(NCORES):
        in_maps.append({
            "x": x[c],
            "h_prev": hp[c],
            "c_prevT": cpT[c],
            "Wg": Wg,
            "bg": bg_dev,
            "WhyT": Why_dev,
            "by_col": by_dev,
        })

    trace = os.environ.get("LSTM_TRACE", "0") == "1"
    res = run_bass_kernel_spmd(nc, in_maps, core_ids=list(range(NCORES)), trace=trace)
    _last_results = res

    y_t = np.empty((B, OUT), np.float32)
    c_t = np.empty((B, H), np.float32)
    h_t = np.empty((B, H), np.float32)
    for c in range(NCORES):
        r = res.results[c]
        y_t[c * BC:(c + 1) * BC] = r["y_tT"].T
        c_t[c * BC:(c + 1) * BC] = r["c_tT"].T
        h_t[c * BC:(c + 1) * BC] = r["h_tT"].T
    return (y_t, c_t, h_t)


# revision 8
# speedup vs baseline: 1.2580x; 1.2580x over previous
"""Bass/Trainium2 kernel for nn_BasicLSTM: LayerNorm-LSTM cell, B=16384, dims 1024.

Strategy (data-parallel over 8 cores, batch-sharded, weights replicated):
  Per core: batch 2048. Feature-major gate computation:
    - LN(x), LN(h_prev) computed batch-major on device (DVE free-dim reduce),
      normalized output cast to fp16 and DMA-transposed into actT [feat, batch].
      LN scale/bias are folded into the gate weights/biases on the host.
    - Gate matmuls: out[gate_feat 128, batch 512] = WgT_tile.T @ actT_tile,
      fp16 x fp16, K=2048 (16 k-tiles), PSUM fp32 accumulation.
      Per-gate-feature biases fused into the ACT nonlinearity (per-partition bias).
    - Elementwise c_t/h_t in feature-major layout (c_prev host-transposed).
    - y matmul from fp16 h_t (feature-major already), erf-Gelu via ACT.
    - Outputs written feature-major [1024, 2048]; host transposes back.
"""
import os
import numpy as np

B, IN, H, OUT = 16384, 1024, 1024, 1024
NCORES = 8
BC = B // NCORES          # 2048 rows per core
EPS = 1e-5
P = 128
KT_G = 16                 # gate contraction k-tiles (2048 / 128)
KT_Y = 8                  # y contraction k-tiles (1024 / 128)
NB4 = BC // 512           # batch 512-blocks per core = 4
NOC = 8                   # gate-feature 128-chunks = 1024/128

# Use erf-composed gelu unless env overrides (ACT Gelu table may be approximate).
GELU_ERF = os.environ.get("LSTM_GELU_ERF", "1") == "1"

_CACHE = {}

_last_results = None  # BassKernelResults of the most recent run (for test harness)


def _build_nc():
    import concourse.tile as tile
    from concourse import bacc, mybir
    from concourse._compat import get_trn_type

    f32 = mybir.dt.float32
    f16 = mybir.dt.float16
    AF = mybir.ActivationFunctionType
    ALU = mybir.AluOpType

    nc = bacc.Bacc(get_trn_type() or "TRN2", target_bir_lowering=False)

    x_d = nc.dram_tensor("x", (BC, IN), f32, kind="ExternalInput")
    h_d = nc.dram_tensor("h_prev", (BC, H), f32, kind="ExternalInput")
    cT_d = nc.dram_tensor("c_prevT", (H, BC), f32, kind="ExternalInput")
    wg_d = nc.dram_tensor("Wg", (NOC, KT_G, P, 512), f16, kind="ExternalInput")
    bg_d = nc.dram_tensor("bg", (P, 32), f32, kind="ExternalInput")
    why_d = nc.dram_tensor("WhyT", (KT_Y, P, OUT), f16, kind="ExternalInput")
    by_d = nc.dram_tensor("by_col", (P, 8), f32, kind="ExternalInput")

    ct_o = nc.dram_tensor("c_tT", (H, BC), f32, kind="ExternalOutput")
    ht_o = nc.dram_tensor("h_tT", (H, BC), f32, kind="ExternalOutput")
    yt_o = nc.dram_tensor("y_tT", (OUT, BC), f32, kind="ExternalOutput")

    with tile.TileContext(nc) as tc:
        with (
            tc.tile_pool(name="const", bufs=1) as const,
            tc.tile_pool(name="actT", bufs=NB4) as actT_pool,
            tc.tile_pool(name="h16", bufs=NB4) as h16_pool,
            tc.tile_pool(name="wg", bufs=2) as wg_pool,
            tc.tile_pool(name="why", bufs=1) as why_pool,
            tc.tile_pool(name="xh", bufs=4) as xh_pool,
            tc.tile_pool(name="sqp", bufs=1) as sq_pool,
            tc.tile_pool(name="lnst", bufs=6) as lnst_pool,
            tc.tile_pool(name="xn16", bufs=3) as xn16_pool,
            tc.tile_pool(name="gate", bufs=2) as gate_pool,
            tc.tile_pool(name="cpv", bufs=2) as cpv_pool,
            tc.tile_pool(name="outs", bufs=4) as outs_pool,
            tc.tile_pool(name="zdram", bufs=1, space="DRAM") as zdram_pool,
            tc.tile_pool(name="ps", bufs=8, space="PSUM") as ps_pool,
        ):
            bg_sb = const.tile([P, 32], f32)
            nc.sync.dma_start(bg_sb[:], bg_d[:])
            by_sb = const.tile([P, 8], f32)
            nc.sync.dma_start(by_sb[:], by_d[:])

            # Resident fp16 transposed activations: 4 b4-blocks of [feat_p, k, batch512]
            actT = [actT_pool.tile([P, KT_G, 512], f16, tag="actT", name=f"actT{i}") for i in range(NB4)]
            # Resident fp16 feature-major h_t for the y matmul
            h16 = [h16_pool.tile([P, KT_Y, 512], f16, tag="h16", name=f"h16_{i}") for i in range(NB4)]

            # DRAM scratch for LN outputs (batch-major fp16), transposed per-b4 below
            zx_dram = zdram_pool.tile([BC, 1024], f16, name="zx_dram")
            zh_dram = zdram_pool.tile([BC, 1024], f16, name="zh_dram")

            # ---- Phase 1: LN(x), LN(h_prev) batch-major; cast fp16; store to DRAM ----
            def ln_panel(src_d, m, ko_off):
                xp = xh_pool.tile([P, 1024], f32, tag="xh")
                nc.gpsimd.dma_start(xp[:], src_d[m * P:(m + 1) * P, :])
                s1 = lnst_pool.tile([P, 1], f32, tag="s1")
                s2 = lnst_pool.tile([P, 1], f32, tag="s2")
                sq = sq_pool.tile([P, 1024], f32, tag="sq")
                nc.scalar.activation(sq[:], xp[:], AF.Square, accum_out=s2[:])
                nc.scalar.activation(sq[:], xp[:], AF.Identity, accum_out=s1[:])
                mu = lnst_pool.tile([P, 1], f32, tag="mu")
                nc.vector.tensor_scalar_mul(mu[:], s1[:], 1.0 / 1024.0)
                ve = lnst_pool.tile([P, 1], f32, tag="ve")
                nc.vector.tensor_scalar(ve[:], s2[:], 1.0 / 1024.0, EPS, ALU.mult, ALU.add)
                m2 = lnst_pool.tile([P, 1], f32, tag="m2")
                nc.vector.tensor_mul(out=m2[:], in0=mu[:], in1=mu[:])
                nc.vector.tensor_tensor(ve[:], ve[:], m2[:], ALU.subtract)
                sd = lnst_pool.tile([P, 1], f32, tag="sd")
                nc.scalar.sqrt(sd[:], ve[:])
                rstd = lnst_pool.tile([P, 1], f32, tag="rstd")
                nc.vector.reciprocal(rstd[:], sd[:])
                mur = lnst_pool.tile([P, 1], f32, tag="mur")
                nc.vector.tensor_mul(out=mur[:], in0=mu[:], in1=rstd[:])
                z16 = xn16_pool.tile([P, 1024], f16, tag="z16")
                nc.vector.tensor_scalar(z16[:], xp[:], rstd[:, 0:1], mur[:, 0:1],
                                        ALU.mult, ALU.subtract)
                zdst = zx_dram if ko_off == 0 else zh_dram
                nc.sync.dma_start(zdst[m * P:(m + 1) * P, :], z16[:])

            for b4 in range(NB4):
                for mloc in range(4):
                    ln_panel(x_d, b4 * 4 + mloc, 0)
                # x-half of this b4 is in DRAM: one big contiguous-dest transpose
                nc.sync.dma_start_transpose(
                    actT[b4][:, 0:8, :], zx_dram[b4 * 512:(b4 + 1) * 512, :])
                for mloc in range(4):
                    ln_panel(h_d, b4 * 4 + mloc, 8)
                nc.sync.dma_start_transpose(
                    actT[b4][:, 8:16, :], zh_dram[b4 * 512:(b4 + 1) * 512, :])

            # ---- Phase 2: gates (oc-major, weights streamed once) ----
            why = why_pool.tile([P, KT_Y, 1024], f16, tag="why")
            nc.gpsimd.dma_start(why[:], why_d.rearrange("ko p c -> p ko c"))
            for oc in range(NOC):
                wg = wg_pool.tile([P, KT_G, 512], f16, tag="wg")
                nc.gpsimd.dma_start(wg[:], wg_d[oc].rearrange("ko p c -> p ko c"))
                for b4 in range(NB4):
                    pss = []
                    for g in range(4):
                        ps = ps_pool.tile([P, 512], f32, tag="ps")
                        for k in range(KT_G):
                            nc.tensor.matmul(ps[:], wg[:, k, g * P:(g + 1) * P],
                                             actT[b4][:, k, :],
                                             start=(k == 0), stop=(k == KT_G - 1))
                        pss.append(ps)
                    f_sb = gate_pool.tile([P, 512], f32, tag="f")
                    nc.scalar.activation(f_sb[:], pss[0][:], AF.Sigmoid,
                                         bias=bg_sb[:, oc * 4 + 0:oc * 4 + 1])
                    i_sb = gate_pool.tile([P, 512], f32, tag="i")
                    nc.scalar.activation(i_sb[:], pss[1][:], AF.Sigmoid,
                                         bias=bg_sb[:, oc * 4 + 1:oc * 4 + 2])
                    o_sb = gate_pool.tile([P, 512], f32, tag="o")
                    nc.scalar.activation(o_sb[:], pss[2][:], AF.Sigmoid,
                                         bias=bg_sb[:, oc * 4 + 2:oc * 4 + 3])
                    cp_sb = gate_pool.tile([P, 512], f32, tag="cp")
                    nc.scalar.activation(cp_sb[:], pss[3][:], AF.Tanh,
                                         bias=bg_sb[:, oc * 4 + 3:oc * 4 + 4])

                    cpv = cpv_pool.tile([P, 512], f32, tag="cpv")
                    nc.scalar.dma_start(cpv[:], cT_d[oc * P:(oc + 1) * P,
                                                     b4 * 512:(b4 + 1) * 512])
                    nc.vector.tensor_mul(out=f_sb[:], in0=f_sb[:], in1=cpv[:])
                    nc.vector.tensor_mul(out=i_sb[:], in0=i_sb[:], in1=cp_sb[:])
                    ct = outs_pool.tile([P, 512], f32, tag="ct")
                    nc.vector.tensor_add(out=ct[:], in0=f_sb[:], in1=i_sb[:])
                    nc.sync.dma_start(ct_o[oc * P:(oc + 1) * P,
                                           b4 * 512:(b4 + 1) * 512], ct[:])
                    nc.scalar.activation(cp_sb[:], ct[:], AF.Tanh)
                    nc.vector.tensor_mul(out=o_sb[:], in0=o_sb[:], in1=cp_sb[:])
                    nc.sync.dma_start(ht_o[oc * P:(oc + 1) * P,
                                           b4 * 512:(b4 + 1) * 512], o_sb[:])
                    nc.vector.tensor_copy(h16[b4][:, oc, :], o_sb[:])

            # ---- Phase 3: y ----
            for b4 in range(NB4):
                for n in range(OUT // P):
                    ps = ps_pool.tile([P, 512], f32, tag="ps")
                    for k in range(KT_Y):
                        nc.tensor.matmul(ps[:], why[:, k, n * P:(n + 1) * P],
                                         h16[b4][:, k, :],
                                         start=(k == 0), stop=(k == KT_Y - 1))
                    yt = outs_pool.tile([P, 512], f32, tag="yt")
                    u = gate_pool.tile([P, 512], f32, tag="f", name="u")
                    nc.scalar.activation(u[:], ps[:], AF.Identity,
                                         bias=by_sb[:, n:n + 1])
                    e = gate_pool.tile([P, 512], f32, tag="i", name="e")
                    nc.scalar.activation(e[:], u[:], AF.Erf,
                                         scale=float(1.0 / np.sqrt(2.0)))
                    nc.vector.tensor_scalar(e[:], e[:], 0.5, 0.5, ALU.mult, ALU.add)
                    nc.vector.tensor_mul(out=yt[:], in0=u[:], in1=e[:])
                    nc.sync.dma_start(yt_o[n * P:(n + 1) * P,
                                           b4 * 512:(b4 + 1) * 512], yt[:])

    nc.compile()
    return nc


def _host_prep(inputs):
    """Build replicated device-side weight/bias arrays from the full inputs."""
    f32 = np.float32
    ln_w = inputs["ln_w"].astype(f32)
    ln_b = inputs["ln_b"].astype(f32)
    lnh_w = inputs["lnh_w"].astype(f32)
    lnh_b = inputs["lnh_b"].astype(f32)

    Wx = np.concatenate([inputs["Wxf"], inputs["Wxi"], inputs["Wxo"], inputs["Wxc"]],
                        axis=0).astype(f32)          # [4096, IN] (out, in)
    Wh = np.concatenate([inputs["Whf"], inputs["Whi"], inputs["Who"], inputs["Whc"]],
                        axis=0).astype(f32)          # [4096, H]
    bg = np.concatenate([inputs["bf"], inputs["bi"], inputs["bo"], inputs["bc"]]).astype(f32)

    # Fold LN affine into weights / bias:
    #   xn @ Wx.T = z_x @ (ln_w * Wx).T + ln_b @ Wx.T
    Wx_eff = Wx * ln_w[None, :]
    Wh_eff = Wh * lnh_w[None, :]
    bias_eff = bg + Wx @ ln_b + Wh @ lnh_b           # [4096]

    Wcat = np.concatenate([Wx_eff.T, Wh_eff.T], axis=0)  # [2048 in', 4096 out]
    # Device layout [oc, ko, p, (g,j)] with 512-col groups [f_oc|i_oc|o_oc|c_oc]
    Wg = (Wcat.reshape(KT_G, P, 4, NOC, P)
              .transpose(3, 0, 1, 2, 4)
              .reshape(NOC, KT_G, P, 512)
              .astype(np.float16))
    bg_dev = np.ascontiguousarray(
        bias_eff.reshape(4, NOC, P).transpose(2, 1, 0).reshape(P, 32)).astype(f32)

    WhyT = np.ascontiguousarray(inputs["Why"].astype(f32).T)   # [H, OUT]
    Why_dev = WhyT.reshape(KT_Y, P, OUT).astype(np.float16)
    by_dev = np.ascontiguousarray(inputs["by"].astype(f32).reshape(8, P).T)  # [P, 8]

    return Wg, bg_dev, Why_dev, by_dev


def kernel(**inputs):
    global _last_results
    from concourse.bass_utils import run_bass_kernel_spmd

    if "nc" not in _CACHE:
        _CACHE["nc"] = _build_nc()
    nc = _CACHE["nc"]

    Wg, bg_dev, Why_dev, by_dev = _host_prep(inputs)

    x = np.ascontiguousarray(inputs["x"], dtype=np.float32).reshape(NCORES, BC, IN)
    hp = np.ascontiguousarray(inputs["h_prev"], dtype=np.float32).reshape(NCORES, BC, H)
    cpT = np.ascontiguousarray(
        np.asarray(inputs["c_prev"], dtype=np.float32).reshape(NCORES, BC, H)
        .transpose(0, 2, 1))                          # [NCORES, H, BC]

    in_maps = []
    for c in range(NCORES):
        in_maps.append({
            "x": x[c],
            "h_prev": hp[c],
            "c_prevT": cpT[c],
            "Wg": Wg,
            "bg": bg_dev,
            "WhyT": Why_dev,
            "by_col": by_dev,
        })

    trace = os.environ.get("LSTM_TRACE", "0") == "1"
    res = run_bass_kernel_spmd(nc, in_maps, core_ids=list(range(NCORES)), trace=trace)
    _last_results = res

    y_t = np.empty((B, OUT), np.float32)
    c_t = np.empty((B, H), np.float32)
    h_t = np.empty((B, H), np.float32)
    for c in range(NCORES):
        r = res.results[c]
        y_t[c * BC:(c + 1) * BC] = r["y_tT"].T
        c_t[c * BC:(c + 1) * BC] = r["c_tT"].T
        h_t[c * BC:(c + 1) * BC] = r["h_tT"].T
    return (y_t, c_t, h_t)
